# revision 42
# baseline (speedup 1.0000x reference)
"""Trainium2 Bass kernel for a differentiable addressing head (NTM-style).

Computes, for each batch b:
    key   = cs @ Wk;  beta = softplus(cs@Wb+bb)+1;  gate = sigmoid(cs@Wg+bg)
    shift = softmax(cs@Ws+bs);  gamma = softplus(cs@Wgam+bgam)+1
    sim   = (key . mem[n]) / (|key||mem[n]| + eps)
    cw    = softmax(beta * sim);  g = gate*cw + (1-gate)*pw
    sh    = circular_conv(g, shift);  w = (sh+1e-8)^gamma / (sum + eps)

Sharding: data-parallel over batch across 8 cores (8 batches/core).
Heavy phase is one pass over memory (32 MB/core) computing dot products and
row norms on the PE; everything else is O(B*N) light work.

Self-contained: hardcodes shapes B=64, N=8192, D=128, C=256.
"""

import os
import sys

import numpy as np

for _p in ("/opt/trn_rl_repo", "/opt/pypackages"):
    if _p not in sys.path and os.path.isdir(_p):
        sys.path.insert(0, _p)

import concourse.bacc as bacc
import concourse.bass as bass
import concourse.tile as tile
from concourse import mybir
from concourse.bass_utils import run_bass_kernel_spmd

F32 = mybir.dt.float32
F32R = mybir.dt.float32r  # single-pass truncated-precision PE mode (4x fp32)
BF16 = mybir.dt.bfloat16
AF = mybir.ActivationFunctionType
OP = mybir.AluOpType

B, N, D, C = 64, 8192, 128, 256
NCORES = 8
BL = B // NCORES          # batches per core = 8
NW = 16                   # 512-wide windows per batch row
W = N // NW               # 512, window width (= light-tile free dim)
NST = 2                   # DMA stripes per batch
SW = N // NST             # 2048, stripe width
EPS = 1e-8

# Module-level caches so repeat calls don't rebuild/recompile.
_NC = None
PROFILE = False
LAST_RESULTS = None       # BassKernelResults of the last run (for profiling)


def _consts():
    ind = np.zeros((128, BL), np.float32)        # ind[p, b] = 1 iff p//16 == b
    for p in range(128):
        ind[p, p // NW] = 1.0
    indT = np.ascontiguousarray(ind.T)           # (8, 128)
    ones_strip = np.zeros((128, 255), np.float32)  # ones column at col 127
    ones_strip[:, 127] = 1.0
    return ind, indT, ones_strip


def build_nc():
    # Bacc (not plain Bass): its compile() runs generate_event_semaphores,
    # which splits multi-wait sync conditions to satisfy the TRN2 limit of
    # one sync wait per instruction.
    nc = bacc.Bacc()
    ind_np, indT_np, ones_strip_np = _consts()

    memT_d = nc.dram_tensor("memT", [BL, D, N], BF16, kind="ExternalInput")
    csT_d = nc.dram_tensor("csT", [C, BL], F32, kind="ExternalInput")
    pw_d = nc.dram_tensor("pw", [BL, N], F32, kind="ExternalInput")
    Wk_d = nc.dram_tensor("Wk", [C, D], F32, kind="ExternalInput")
    Wcat_d = nc.dram_tensor("Wcat", [C, 6], F32, kind="ExternalInput")
    bias6_d = nc.dram_tensor("bias6", [BL, 6], F32, kind="ExternalInput")
    out_d = nc.dram_tensor("out", [BL, N], F32, kind="ExternalOutput")

    ident_c = nc.inline_tensor(np.eye(128, dtype=np.float32), "ident_c")
    ind_c = nc.inline_tensor(ind_np, "ind_c")
    indT_c = nc.inline_tensor(indT_np, "indT_c")
    ones_col_c = nc.inline_tensor(np.ones((128, 1), np.float32), "ones_col_c")
    one1_c = nc.inline_tensor(np.ones((1, 1), np.float32), "one1_c")
    ones_strip_c = nc.inline_tensor(ones_strip_np, "ones_strip_c")
    eps_col_c = nc.inline_tensor(np.full((128, 1), EPS, np.float32), "eps_col_c")

    with tile.TileContext(nc) as tc:
        with (
            tc.tile_pool(name="const", bufs=1) as cp,
            tc.tile_pool(name="mem", bufs=6) as memp,
            tc.tile_pool(name="light", bufs=1) as lp,
            tc.tile_pool(name="psmm", bufs=2, space="PSUM") as psA,
            tc.tile_pool(name="pstiny", bufs=3, space="PSUM") as psB,
        ):
            # ---- constants / small inputs to SBUF ----
            # Self-loading fp32 matmuls only support ONE sync wait, and
            # Tile's vector clock is not transitive across procs. So every
            # tile a matmul reads is staged through GPSIMD (one producer
            # proc); once PE has waited on the newest GPSIMD tick, all
            # const reads are free.
            def dma_then_g(name, shape, src_ap, dtype=F32):
                raw = cp.tile(shape, F32, name=f"{name}_raw")
                nc.sync.dma_start(raw[:], src_ap)
                g = cp.tile(shape, dtype, name=name)
                nc.gpsimd.tensor_copy(g[:], raw[:])
                return g

            csT0 = dma_then_g("csT0", [128, BL], csT_d[:][0:128, :])
            csT1 = dma_then_g("csT1", [128, BL], csT_d[:][128:256, :])
            Wk0 = dma_then_g("Wk0", [128, D], Wk_d[:][0:128, :])
            Wk1 = dma_then_g("Wk1", [128, D], Wk_d[:][128:256, :])
            memT_ap = memT_d[:]
            stripes = {}
            for s in range(NST):
                st = memp.tile([128, SW], BF16, tag="mst", name=f"mst_0_{s}")
                nc.sync.dma_start(st[:], memT_ap[0][:, s * SW : (s + 1) * SW])
                stripes[(0, s)] = st
            Wc0 = dma_then_g("Wc0", [128, 6], Wcat_d[:][0:128, :])
            Wc1 = dma_then_g("Wc1", [128, 6], Wcat_d[:][128:256, :])
            ident = dma_then_g("ident", [128, 128], ident_c[:])
            ind_sb = dma_then_g("ind_sb", [128, BL], ind_c[:])
            indT_sb = dma_then_g("indT_sb", [BL, 128], indT_c[:])
            ones_col = dma_then_g("ones_col", [128, 1], ones_col_c[:])
            one1 = dma_then_g("one1", [1, 1], one1_c[:])
            ones_strip = dma_then_g("ones_strip", [128, 255], ones_strip_c[:], dtype=BF16)
            bias6 = cp.tile([BL, 6], F32)
            nc.sync.dma_start(bias6[:], bias6_d[:])
            eps_col = cp.tile([128, 1], F32)
            nc.sync.dma_start(eps_col[:], eps_col_c[:])
            pw_raw = cp.tile([128, W], F32)
            nc.sync.dma_start(pw_raw[:], pw_d[:].rearrange("b (q f) -> (b q) f", f=W))
            pw_sb = cp.tile([128, W], F32)
            nc.vector.tensor_copy(pw_sb[:], pw_raw[:])

            # ---- projections: key_T (D, BL) and proj (BL, 6) ----
            key_ps = psB.tile([128, BL], F32, tag="tiny")
            nc.tensor.matmul(key_ps[:], lhsT=Wk0[:], rhs=csT0[:], start=True, stop=False)
            nc.tensor.matmul(key_ps[:], lhsT=Wk1[:], rhs=csT1[:], start=False, stop=True)
            keyT = cp.tile([128, BL], F32)
            nc.vector.tensor_copy(keyT[:], key_ps[:])

            proj_ps = psB.tile([BL, 6], F32, tag="tiny")
            nc.tensor.matmul(proj_ps[:], lhsT=csT0[:], rhs=Wc0[:], start=True, stop=False)
            nc.tensor.matmul(proj_ps[:], lhsT=csT1[:], rhs=Wc1[:], start=False, stop=True)
            proj = lp.tile([BL, 6], F32)
            nc.vector.tensor_add(proj[:], proj_ps[:], bias6[:])

            # zero-padded key strips: strips[:, b, 31] = key_T[:, b]
            # (all on DVE: gpsimd runs on parallel Q7 lanes, so mixed-engine
            # writers would force multi-wait copies, which walrus rejects)
            strips = cp.tile([128, BL, 255], BF16)
            nc.vector.memset(strips[:], 0.0)
            for b in range(BL):
                nc.vector.tensor_copy(strips[:, b, 127:128], keyT[:, b : b + 1])

            # Absorber matmuls: advance PE's observed GPSIMD tick past every
            # staged constant so later matmuls carry at most one new wait
            # (self-loading fp32 matmuls support only a single sync wait).
            # The output tile is write-only and never rotated, so its slot
            # release never puts a wait on another instruction.
            absorb = psB.tile([128, 8], F32, tag="absorb", bufs=1, name="absorb")
            nc.tensor.matmul(absorb[:, 0:1], lhsT=ident[:], rhs=ones_col[:],
                             start=True, stop=True, skip_group_check=True)
            nc.tensor.matmul(absorb[0:8, 1:2], lhsT=ind_sb[:], rhs=ones_col[:],
                             start=True, stop=True, skip_group_check=True)
            nc.tensor.matmul(absorb[0:128, 2:3], lhsT=indT_sb[:], rhs=indT_sb[:, 0:1],
                             start=True, stop=True, skip_group_check=True)
            nc.tensor.matmul(absorb[0:1, 3:4], lhsT=one1[:], rhs=one1[:],
                             start=True, stop=True, skip_group_check=True)
            nc.tensor.matmul(absorb[0:128, 4:6], lhsT=ones_strip[:, 0:128], rhs=ones_strip[:, 126:128],
                             start=True, stop=True, skip_group_check=True)
            nc.tensor.matmul(absorb[0:8, 6:8], lhsT=strips[:, :, 127], rhs=strips[:, 0, 126:128],
                             start=True, stop=True, skip_group_check=True)

            # |key|^2 per batch -> (BL, 1)
            kq = lp.tile([128, BL], F32)
            nc.scalar.activation(kq[:], keyT[:], AF.Square)
            kn2_ps = psB.tile([BL, 1], F32, tag="tiny")
            nc.tensor.matmul(kn2_ps[:], lhsT=kq[:], rhs=ones_col[:], start=True, stop=True)
            kn2 = lp.tile([BL, 1], F32)
            nc.vector.tensor_copy(kn2[:], kn2_ps[:])
            kn2F_ps = psB.tile([128, 1], F32, tag="tiny")
            nc.tensor.matmul(kn2F_ps[:], lhsT=indT_sb[:], rhs=kn2[:], start=True, stop=True)
            F_kn2 = lp.tile([128, 1], F32)
            nc.vector.tensor_copy(F_kn2[:], kn2F_ps[:])

            # ---- heavy phase: dot[b, n] and normsq[b, n] ----
            D_sb = lp.tile([128, W], F32)    # dot, light layout (p = b*16+t, f)
            NS_sb = lp.tile([128, W], F32)   # |mem|^2, light layout
            # Per stripe: 4 dot matmuls read the raw stripe, then the stripe
            # is squared IN PLACE (slot-reuse waits land on the exempt DMA
            # instruction), then the PREVIOUS stripe's 4 norm matmuls run —
            # one-stripe software pipeline so the PE never waits on a square.
            # A tiny fresh-tile "toucher" copy absorbs the stripe's DMA tick
            # into the squaring engine's clock first, keeping every compute
            # instruction at <=1 sync wait (walrus codegen limit).
            pending = None
            dotP = psA.tile([128, W], F32, tag="dotP", bufs=1, name="dotP")
            nrmP = psA.tile([128, W], F32, tag="nrmP", bufs=1, name="nrmP")

            def emit_nrms(p):
                pb, ps, pst = p
                for tl in range(SW // W):
                    t = ps * (SW // W) + tl
                    row = pb * NW + t
                    nc.tensor.matmul(
                        nrmP[:, :],
                        lhsT=ones_strip[:, 127 - row : 255 - row],
                        rhs=pst[:, tl * W : (tl + 1) * W],
                        start=(row == 0),
                        stop=(row == 127),
                        skip_group_check=True,
                    )
                if (pb == BL - 1) and (ps == NST - 1):
                    nc.vector.tensor_copy(D_sb[:, :], dotP[:, :])
                    nc.vector.tensor_copy(NS_sb[:, :], nrmP[:, :])

            for b in range(BL):
                for s in range(NST):
                    if (b, s) in stripes:
                        st = stripes[(b, s)]
                    else:
                        st = memp.tile([128, SW], BF16, tag="mst", name=f"mst_{b}_{s}")
                        nc.sync.dma_start(st[:], memT_ap[b][:, s * SW : (s + 1) * SW])
                    for tl in range(SW // W):
                        t = s * (SW // W) + tl
                        row = b * NW + t
                        nc.tensor.matmul(
                            dotP[:, :],
                            lhsT=strips[:, b, 127 - row : 255 - row],
                            rhs=st[:, tl * W : (tl + 1) * W],
                            start=(row == 0),
                            stop=(row == 127),
                            skip_group_check=True,
                        )
                    tch = cp.tile([128, 1], F32, name=f"tch_{b}_{s}")
                    if s % 2 == 0:
                        nc.scalar.copy(tch[:], st[:, 0:1])
                        nc.scalar.activation(st[:], st[:], AF.Square)
                    else:
                        nc.vector.tensor_copy(tch[:], st[:, 0:1])
                        nc.vector.tensor_mul(st[:], st[:], st[:])
                    if pending is not None:
                        emit_nrms(pending)
                    pending = (b, s, st)
            emit_nrms(pending)

            # ---- light phase ----
            # 1/(kn*mn) = exp(-0.5*ln(kn2*ns)); ln/exp LUTs are accurate
            # (~1e-5 rel) and stay in the natural_log_exp table set, so the
            # whole kernel uses ONE ACT table set (square/copy are in all).
            Lv = lp.tile([128, W], F32)
            nc.scalar.activation(Lv[:], NS_sb[:], AF.Ln, scale=F_kn2[:])
            y1 = lp.tile([128, W], F32)
            nc.scalar.activation(y1[:], Lv[:], AF.Exp, scale=-0.5)
            sim = lp.tile([128, W], F32)
            nc.vector.tensor_mul(sim[:], D_sb[:], y1[:])

            # per-batch scalars: beta, 1-gate, s0, s1, s2, gamma  (BL, 6)
            # projX = proj + 0*NS: artificial dep so the scheduler cannot
            # hoist these ACT transcendentals into the heavy-square stream
            # (that would thrash the ACT table set between squares)
            projX = lp.tile([BL, 6], F32)
            nc.vector.scalar_tensor_tensor(
                projX[:], NS_sb[0:BL, 0:6], 0.0, proj[:], op0=OP.mult, op1=OP.add
            )
            proj = projX
            scal = lp.tile([BL, 6], F32)
            # softplus(x) = ln(1 + exp(x)); beta = softplus + 1
            eb = lp.tile([BL, 1], F32)
            nc.scalar.activation(eb[:], proj[:, 0:1], AF.Exp)
            sp_b = lp.tile([BL, 1], F32)
            nc.scalar.activation(sp_b[:], eb[:], AF.Ln, bias=1.0)
            nc.vector.tensor_scalar_add(scal[:, 0:1], sp_b[:], 1.0)
            # gate = sigmoid(x) = 1 / (1 + exp(-x))
            eg = lp.tile([BL, 1], F32)
            nc.scalar.activation(eg[:], proj[:, 1:2], AF.Exp, scale=-1.0)
            dg = lp.tile([BL, 1], F32)
            nc.vector.tensor_scalar_add(dg[:], eg[:], 1.0)
            gate = lp.tile([BL, 1], F32)
            nc.vector.reciprocal(gate[:], dg[:])
            nc.vector.tensor_scalar(
                scal[:, 1:2], gate[:], -1.0, 1.0, op0=OP.mult, op1=OP.add
            )
            e3 = lp.tile([BL, 3], F32)
            nc.scalar.activation(e3[:], proj[:, 2:5], AF.Exp)
            ssum = lp.tile([BL, 1], F32)
            nc.vector.reduce_sum(ssum[:], e3[:], axis=mybir.AxisListType.X)
            rssum = lp.tile([BL, 1], F32)
            nc.vector.reciprocal(rssum[:], ssum[:])
            # on ACT: e3 is ACT-made, rssum DVE-made; a DVE tensor_scalar
            # would need two sync waits (TS struct supports one). Copy back
            # via DVE so scal stays single-producer for the FB matmul.
            sh3 = lp.tile([BL, 3], F32)
            nc.scalar.mul(sh3[:], e3[:], rssum[:])
            nc.vector.tensor_copy(scal[:, 2:5], sh3[:])
            # gamma = softplus(z) + 1 = ln(1 + exp(z)) + 1
            egm = lp.tile([BL, 1], F32)
            nc.scalar.activation(egm[:], proj[:, 5:6], AF.Exp)
            sp_g = lp.tile([BL, 1], F32)
            nc.scalar.activation(sp_g[:], egm[:], AF.Ln, bias=1.0)
            nc.vector.tensor_scalar_add(scal[:, 5:6], sp_g[:], 1.0)
            # broadcast to per-partition fields (128, 6)
            FB_ps = psB.tile([128, 6], F32, tag="tiny")
            nc.tensor.matmul(FB_ps[:], lhsT=indT_sb[:], rhs=scal[:], start=True, stop=True)
            FB = lp.tile([128, 6], F32)
            nc.vector.tensor_copy(FB[:], FB_ps[:])
            F_beta = FB[:, 0:1]
            F_g1 = FB[:, 1:2]
            F_s0 = FB[:, 2:3]
            F_s1 = FB[:, 3:4]
            F_s2 = FB[:, 4:5]
            F_gamma = FB[:, 5:6]

            # content weights: E = exp(beta * sim) (no max-sub: |beta*sim| small)
            E = lp.tile([128, W], F32)
            rs1 = lp.tile([128, 1], F32)
            nc.scalar.activation(E[:], sim[:], AF.Exp, scale=F_beta, accum_out=rs1[:])
            S_ps = psB.tile([BL, 1], F32, tag="tiny")
            nc.tensor.matmul(S_ps[:], lhsT=ind_sb[:], rhs=rs1[:], start=True, stop=True)
            Scol = lp.tile([BL, 1], F32)
            nc.vector.tensor_copy(Scol[:], S_ps[:])
            rS = lp.tile([BL, 1], F32)
            nc.vector.reciprocal(rS[:], Scol[:])
            gs = lp.tile([BL, 1], F32)
            nc.vector.tensor_mul(gs[:], gate[:], rS[:])
            F2_ps = psB.tile([128, 1], F32, tag="tiny")
            nc.tensor.matmul(F2_ps[:], lhsT=indT_sb[:], rhs=gs[:], start=True, stop=True)
            F_gs = lp.tile([128, 1], F32)
            nc.vector.tensor_copy(F_gs[:], F2_ps[:])

            # gated = gs*E + (1-gate)*pw   (gs = gate/softmax_sum)
            t4 = lp.tile([128, W], F32)
            nc.vector.tensor_scalar_mul(t4[:], pw_sb[:], F_g1)
            Esc = lp.tile([128, 1], F32)
            nc.vector.tensor_copy(Esc[:], E[:, 0:1])  # DVE observes ACT@E
            G = lp.tile([128, W], F32)
            nc.vector.scalar_tensor_tensor(
                G[:], E[:], F_gs[:], t4[:], op0=OP.mult, op1=OP.add
            )

            # circular conv: SH = s1*G + s0*roll(G,-1) + s2*roll(G,+1)
            SH = lp.tile([128, W], F32)
            nc.vector.tensor_scalar_mul(SH[:], G[:], F_s1)
            nc.vector.scalar_tensor_tensor(
                SH[:, 0 : W - 1], G[:, 1:W], F_s0, SH[:, 0 : W - 1],
                op0=OP.mult, op1=OP.add,
            )
            nc.vector.scalar_tensor_tensor(
                SH[:, 1:W], G[:, 0 : W - 1], F_s2, SH[:, 1:W],
                op0=OP.mult, op1=OP.add,
            )
            # boundary columns via PE transpose (partition shift is not a DVE op)
            # left-shift boundary: SH[p, W-1] += s0 * G[p+1 (wrap in batch), 0]
            rowL_ps = psB.tile([1, 128], F32, tag="tiny")
            nc.tensor.matmul(rowL_ps[:], lhsT=G[:, 0:1], rhs=ident[:], start=True, stop=True)
            rowL = lp.tile([1, 128], F32)
            nc.vector.tensor_copy(rowL[:], rowL_ps[:])
            rowLs = lp.tile([1, 128], F32)
            nc.vector.tensor_copy(rowLs[:, 0:127], rowL[:, 1:128])
            rL_v = rowL.rearrange("o (g s) -> o g s", s=16)
            rLs_v = rowLs.rearrange("o (g s) -> o g s", s=16)
            nc.vector.tensor_copy(rLs_v[:, :, 15:16], rL_v[:, :, 0:1])
            bl_ps = psB.tile([128, 1], F32, tag="tiny")
            nc.tensor.matmul(bl_ps[:], lhsT=rowLs[:], rhs=one1[:], start=True, stop=True)
            bl = lp.tile([128, 1], F32)
            nc.vector.tensor_copy(bl[:], bl_ps[:])
            nc.vector.scalar_tensor_tensor(
                SH[:, W - 1 : W], bl[:], F_s0, SH[:, W - 1 : W],
                op0=OP.mult, op1=OP.add,
            )
            # right-shift boundary: SH[p, 0] += s2 * G[p-1 (wrap in batch), W-1]
            rowR_ps = psB.tile([1, 128], F32, tag="tiny")
            nc.tensor.matmul(rowR_ps[:], lhsT=G[:, W - 1 : W], rhs=ident[:], start=True, stop=True)
            rowR = lp.tile([1, 128], F32)
            nc.vector.tensor_copy(rowR[:], rowR_ps[:])
            rowRs = lp.tile([1, 128], F32)
            nc.vector.tensor_copy(rowRs[:, 1:128], rowR[:, 0:127])
            rR_v = rowR.rearrange("o (g s) -> o g s", s=16)
            rRs_v = rowRs.rearrange("o (g s) -> o g s", s=16)
            nc.vector.tensor_copy(rRs_v[:, :, 0:1], rR_v[:, :, 15:16])
            br_ps = psB.tile([128, 1], F32, tag="tiny")
            nc.tensor.matmul(br_ps[:], lhsT=rowRs[:], rhs=one1[:], start=True, stop=True)
            br = lp.tile([128, 1], F32)
            nc.vector.tensor_copy(br[:], br_ps[:])
            nc.vector.scalar_tensor_tensor(
                SH[:, 0:1], br[:], F_s2, SH[:, 0:1], op0=OP.mult, op1=OP.add
            )

            # sharpening: P2 = (SH + 1e-8)^gamma = exp(gamma * ln(SH + 1e-8))
            Lg = lp.tile([128, W], F32)
            nc.scalar.activation(Lg[:], SH[:], AF.Ln, bias=eps_col[:])
            P2 = lp.tile([128, W], F32)
            rs2 = lp.tile([128, 1], F32)
            nc.scalar.activation(P2[:], Lg[:], AF.Exp, scale=F_gamma, accum_out=rs2[:])
            S2_ps = psB.tile([BL, 1], F32, tag="tiny")
            nc.tensor.matmul(S2_ps[:], lhsT=ind_sb[:], rhs=rs2[:], start=True, stop=True)
            S2 = lp.tile([BL, 1], F32)
            nc.vector.tensor_scalar_add(S2[:], S2_ps[:], EPS)
            r2 = lp.tile([BL, 1], F32)
            nc.vector.reciprocal(r2[:], S2[:])
            F3_ps = psB.tile([128, 1], F32, tag="tiny")
            nc.tensor.matmul(F3_ps[:], lhsT=indT_sb[:], rhs=r2[:], start=True, stop=True)
            F_r2 = lp.tile([128, 1], F32)
            nc.vector.tensor_copy(F_r2[:], F3_ps[:])

            P2sc = lp.tile([128, 1], F32)
            nc.vector.tensor_copy(P2sc[:], P2[:, 0:1])  # DVE observes ACT@P2
            outsb = lp.tile([128, W], F32)
            nc.vector.tensor_scalar_mul(outsb[:], P2[:], F_r2[:])
            nc.sync.dma_start(
                out_d[:].rearrange("b (q f) -> (b q) f", f=W), outsb[:]
            )
    nc.compile()
    return nc


def _get_nc():
    global _NC
    if _NC is None:
        _NC = build_nc()
    return _NC


def _enable_profiling():
    """Install the axon NTFF profile hook; the agent image lacks
    antenv.axon_hooks, so shim it and register the ctypes-based hook."""
    import types

    import concourse.bass_utils as bu

    bu.upload_artifacts = lambda tmpdir: tmpdir  # no artifact bucket here
    try:
        from antenv.axon_hooks import get_axon_ntff_profile_hook  # noqa: F401

        return
    except ImportError:
        pass
    import antenv

    mod = types.ModuleType("antenv.axon_hooks")
    _holder = {}
    mod.set_axon_ntff_profile_hook = lambda h: _holder.__setitem__("h", h)
    mod.get_axon_ntff_profile_hook = lambda: _holder.get("h")
    sys.modules["antenv.axon_hooks"] = mod
    antenv.axon_hooks = mod
    from trn_agent_boot.trn_boot import _ntff_profile_via_ctypes

    mod.set_axon_ntff_profile_hook(
        _ntff_profile_via_ctypes("/opt/axon/libaxon_pjrt.so")
    )


def kernel(**inputs):
    global LAST_RESULTS
    mem = np.ascontiguousarray(np.asarray(inputs["memory"], dtype=np.float32))
    cs = np.ascontiguousarray(np.asarray(inputs["controller_state"], dtype=np.float32))
    pw = np.ascontiguousarray(np.asarray(inputs["previous_weights"], dtype=np.float32))
    Wk = np.ascontiguousarray(np.asarray(inputs["Wk"], dtype=np.float32))
    Wcat = np.ascontiguousarray(
        np.concatenate(
            [
                np.asarray(inputs["Wb"], np.float32),
                np.asarray(inputs["Wg"], np.float32),
                np.asarray(inputs["Ws"], np.float32),
                np.asarray(inputs["Wgam"], np.float32),
            ],
            axis=1,
        )
    )
    brow = np.concatenate(
        [
            np.asarray(inputs["bb"], np.float32),
            np.asarray(inputs["bg"], np.float32),
            np.asarray(inputs["bs"], np.float32),
            np.asarray(inputs["bgam"], np.float32),
        ]
    )
    bias6 = np.ascontiguousarray(np.broadcast_to(brow[None, :], (BL, 6)).astype(np.float32))

    # shard: core c gets batches [c*BL, (c+1)*BL); memory pre-transposed to (BL, D, N)
    memT = np.ascontiguousarray(
        mem.reshape(NCORES, BL, N, D).transpose(0, 1, 3, 2)
    )
    import ml_dtypes
    memT = memT.astype(ml_dtypes.bfloat16)
    csT = np.ascontiguousarray(cs.reshape(NCORES, BL, C).transpose(0, 2, 1))
    pw_sh = pw.reshape(NCORES, BL, N)

    in_maps = [
        {
            "memT": memT[c],
            "csT": csT[c],
            "pw": np.ascontiguousarray(pw_sh[c]),
            "Wk": Wk,
            "Wcat": Wcat,
            "bias6": bias6,
        }
        for c in range(NCORES)
    ]
    nc = _get_nc()
    if PROFILE:
        _enable_profiling()
    res = run_bass_kernel_spmd(nc, in_maps, list(range(NCORES)), trace=PROFILE)
    LAST_RESULTS = res
    out = np.concatenate([r["out"] for r in res.results], axis=0)
    return out.astype(np.float32)


# revision 43
# speedup vs baseline: 1.0677x; 1.0677x over previous
"""Trainium2 Bass kernel for a differentiable addressing head (NTM-style).

Computes, for each batch b:
    key   = cs @ Wk;  beta = softplus(cs@Wb+bb)+1;  gate = sigmoid(cs@Wg+bg)
    shift = softmax(cs@Ws+bs);  gamma = softplus(cs@Wgam+bgam)+1
    sim   = (key . mem[n]) / (|key||mem[n]| + eps)
    cw    = softmax(beta * sim);  g = gate*cw + (1-gate)*pw
    sh    = circular_conv(g, shift);  w = (sh+1e-8)^gamma / (sum + eps)

Sharding: data-parallel over batch across 8 cores (8 batches/core).
Heavy phase is one pass over memory (32 MB/core) computing dot products and
row norms on the PE; everything else is O(B*N) light work.

Self-contained: hardcodes shapes B=64, N=8192, D=128, C=256.
"""

import os
import sys

import numpy as np

for _p in ("/opt/trn_rl_repo", "/opt/pypackages"):
    if _p not in sys.path and os.path.isdir(_p):
        sys.path.insert(0, _p)

import concourse.bacc as bacc
import concourse.bass as bass
import concourse.tile as tile
from concourse import mybir
from concourse.bass_utils import run_bass_kernel_spmd

F32 = mybir.dt.float32
F32R = mybir.dt.float32r  # single-pass truncated-precision PE mode (4x fp32)
BF16 = mybir.dt.bfloat16
AF = mybir.ActivationFunctionType
OP = mybir.AluOpType

B, N, D, C = 64, 8192, 128, 256
NCORES = 8
BL = B // NCORES          # batches per core = 8
NW = 16                   # 512-wide windows per batch row
W = N // NW               # 512, window width (= light-tile free dim)
NST = 2                   # DMA stripes per batch
SW = N // NST             # 2048, stripe width
EPS = 1e-8

# Module-level caches so repeat calls don't rebuild/recompile.
_NC = None
PROFILE = False
LAST_RESULTS = None       # BassKernelResults of the last run (for profiling)


def _consts():
    ind = np.zeros((128, BL), np.float32)        # ind[p, b] = 1 iff p//16 == b
    for p in range(128):
        ind[p, p // NW] = 1.0
    indT = np.ascontiguousarray(ind.T)           # (8, 128)
    ones_strip = np.zeros((128, 63), np.float32)  # ones column at col 31
    ones_strip[:, 31] = 1.0
    return ind, indT, ones_strip


def build_nc():
    # Bacc (not plain Bass): its compile() runs generate_event_semaphores,
    # which splits multi-wait sync conditions to satisfy the TRN2 limit of
    # one sync wait per instruction.
    nc = bacc.Bacc()
    ind_np, indT_np, ones_strip_np = _consts()

    memT_d = nc.dram_tensor("memT", [BL, D, N], BF16, kind="ExternalInput")
    csT_d = nc.dram_tensor("csT", [C, BL], F32, kind="ExternalInput")
    pw_d = nc.dram_tensor("pw", [BL, N], F32, kind="ExternalInput")
    Wk_d = nc.dram_tensor("Wk", [C, D], F32, kind="ExternalInput")
    Wcat_d = nc.dram_tensor("Wcat", [C, 6], F32, kind="ExternalInput")
    bias6_d = nc.dram_tensor("bias6", [BL, 6], F32, kind="ExternalInput")
    out_d = nc.dram_tensor("out", [BL, N], F32, kind="ExternalOutput")

    ident_c = nc.inline_tensor(np.eye(128, dtype=np.float32), "ident_c")
    ind_c = nc.inline_tensor(ind_np, "ind_c")
    indT_c = nc.inline_tensor(indT_np, "indT_c")
    ones_col_c = nc.inline_tensor(np.ones((128, 1), np.float32), "ones_col_c")
    one1_c = nc.inline_tensor(np.ones((1, 1), np.float32), "one1_c")
    ones_strip_c = nc.inline_tensor(ones_strip_np, "ones_strip_c")
    eps_col_c = nc.inline_tensor(np.full((128, 1), EPS, np.float32), "eps_col_c")

    with tile.TileContext(nc) as tc:
        with (
            tc.tile_pool(name="const", bufs=1) as cp,
            tc.tile_pool(name="mem", bufs=6) as memp,
            tc.tile_pool(name="light", bufs=1) as lp,
            tc.tile_pool(name="psmm", bufs=2, space="PSUM") as psA,
            tc.tile_pool(name="pstiny", bufs=3, space="PSUM") as psB,
        ):
            # ---- constants / small inputs to SBUF ----
            # Self-loading fp32 matmuls only support ONE sync wait, and
            # Tile's vector clock is not transitive across procs. So every
            # tile a matmul reads is staged through GPSIMD (one producer
            # proc); once PE has waited on the newest GPSIMD tick, all
            # const reads are free.
            def dma_then_g(name, shape, src_ap, dtype=F32):
                raw = cp.tile(shape, F32, name=f"{name}_raw")
                nc.sync.dma_start(raw[:], src_ap)
                g = cp.tile(shape, dtype, name=name)
                nc.gpsimd.tensor_copy(g[:], raw[:])
                return g

            csT0 = dma_then_g("csT0", [128, BL], csT_d[:][0:128, :])
            csT1 = dma_then_g("csT1", [128, BL], csT_d[:][128:256, :])
            Wk0 = dma_then_g("Wk0", [128, D], Wk_d[:][0:128, :])
            Wk1 = dma_then_g("Wk1", [128, D], Wk_d[:][128:256, :])
            memT_ap = memT_d[:]
            stripes = {}
            for s in range(NST):
                st = memp.tile([128, SW], BF16, tag="mst", name=f"mst_0_{s}")
                nc.sync.dma_start(st[:], memT_ap[0][:, s * SW : (s + 1) * SW])
                stripes[(0, s)] = st
            Wc0 = dma_then_g("Wc0", [128, 6], Wcat_d[:][0:128, :])
            Wc1 = dma_then_g("Wc1", [128, 6], Wcat_d[:][128:256, :])
            ident = dma_then_g("ident", [128, 128], ident_c[:])
            ind_sb = dma_then_g("ind_sb", [128, BL], ind_c[:])
            indT_sb = dma_then_g("indT_sb", [BL, 128], indT_c[:])
            ones_col = dma_then_g("ones_col", [128, 1], ones_col_c[:])
            one1 = dma_then_g("one1", [1, 1], one1_c[:])
            ones_strip = dma_then_g("ones_strip", [128, 63], ones_strip_c[:], dtype=BF16)
            bias6 = cp.tile([BL, 6], F32)
            nc.sync.dma_start(bias6[:], bias6_d[:])
            eps_col = cp.tile([128, 1], F32)
            nc.sync.dma_start(eps_col[:], eps_col_c[:])
            pw_raw = cp.tile([128, W], F32)
            nc.sync.dma_start(pw_raw[:], pw_d[:].rearrange("b (q f) -> (b q) f", f=W))
            pw_sb = cp.tile([128, W], F32)
            nc.vector.tensor_copy(pw_sb[:], pw_raw[:])

            # ---- projections: key_T (D, BL) and proj (BL, 6) ----
            key_ps = psB.tile([128, BL], F32, tag="tiny")
            nc.tensor.matmul(key_ps[:], lhsT=Wk0[:], rhs=csT0[:], start=True, stop=False)
            nc.tensor.matmul(key_ps[:], lhsT=Wk1[:], rhs=csT1[:], start=False, stop=True)
            keyT = cp.tile([128, BL], F32)
            nc.vector.tensor_copy(keyT[:], key_ps[:])

            proj_ps = psB.tile([BL, 6], F32, tag="tiny")
            nc.tensor.matmul(proj_ps[:], lhsT=csT0[:], rhs=Wc0[:], start=True, stop=False)
            nc.tensor.matmul(proj_ps[:], lhsT=csT1[:], rhs=Wc1[:], start=False, stop=True)
            proj = lp.tile([BL, 6], F32)
            nc.vector.tensor_add(proj[:], proj_ps[:], bias6[:])

            # zero-padded key strips: strips[:, b, 31] = key_T[:, b]
            # (all on DVE: gpsimd runs on parallel Q7 lanes, so mixed-engine
            # writers would force multi-wait copies, which walrus rejects)
            strips = cp.tile([128, BL, 63], BF16)
            nc.vector.memset(strips[:], 0.0)
            for b in range(BL):
                nc.vector.tensor_copy(strips[:, b, 31:32], keyT[:, b : b + 1])

            # Absorber matmuls: advance PE's observed GPSIMD tick past every
            # staged constant so later matmuls carry at most one new wait
            # (self-loading fp32 matmuls support only a single sync wait).
            # The output tile is write-only and never rotated, so its slot
            # release never puts a wait on another instruction.
            absorb = psB.tile([128, 8], F32, tag="absorb", bufs=1, name="absorb")
            nc.tensor.matmul(absorb[:, 0:1], lhsT=ident[:], rhs=ones_col[:],
                             start=True, stop=True, skip_group_check=True)
            nc.tensor.matmul(absorb[0:8, 1:2], lhsT=ind_sb[:], rhs=ones_col[:],
                             start=True, stop=True, skip_group_check=True)
            nc.tensor.matmul(absorb[0:128, 2:3], lhsT=indT_sb[:], rhs=indT_sb[:, 0:1],
                             start=True, stop=True, skip_group_check=True)
            nc.tensor.matmul(absorb[0:1, 3:4], lhsT=one1[:], rhs=one1[:],
                             start=True, stop=True, skip_group_check=True)
            nc.tensor.matmul(absorb[0:63, 4:5], lhsT=ones_strip[:], rhs=ones_strip[:, 31:32],
                             start=True, stop=True, skip_group_check=True)
            nc.tensor.matmul(absorb[0:8, 5:6], lhsT=strips[:, :, 31], rhs=strips[:, 0, 31:32],
                             start=True, stop=True, skip_group_check=True)

            # |key|^2 per batch -> (BL, 1)
            kq = lp.tile([128, BL], F32)
            nc.scalar.activation(kq[:], keyT[:], AF.Square)
            kn2_ps = psB.tile([BL, 1], F32, tag="tiny")
            nc.tensor.matmul(kn2_ps[:], lhsT=kq[:], rhs=ones_col[:], start=True, stop=True)
            kn2 = lp.tile([BL, 1], F32)
            nc.vector.tensor_copy(kn2[:], kn2_ps[:])
            kn2F_ps = psB.tile([128, 1], F32, tag="tiny")
            nc.tensor.matmul(kn2F_ps[:], lhsT=indT_sb[:], rhs=kn2[:], start=True, stop=True)
            F_kn2 = lp.tile([128, 1], F32)
            nc.vector.tensor_copy(F_kn2[:], kn2F_ps[:])

            # ---- heavy phase: dot[b, n] and normsq[b, n] ----
            D_sb = lp.tile([128, W], F32)    # dot, light layout (p = b*16+t, f)
            NS_sb = lp.tile([128, W], F32)   # |mem|^2, light layout
            # Per stripe: 4 dot matmuls read the raw stripe, then the stripe
            # is squared IN PLACE (slot-reuse waits land on the exempt DMA
            # instruction), then the PREVIOUS stripe's 4 norm matmuls run —
            # one-stripe software pipeline so the PE never waits on a square.
            # A tiny fresh-tile "toucher" copy absorbs the stripe's DMA tick
            # into the squaring engine's clock first, keeping every compute
            # instruction at <=1 sync wait (walrus codegen limit).
            pending = None
            dotPs = {}
            nrmPs = {}

            def emit_nrms(p):
                pb, ps, pst = p
                pj = pb // 2
                rows = slice(32 * pj, 32 * pj + 32)
                for tl in range(SW // W):
                    t = ps * (SW // W) + tl
                    c = NW * (pb % 2) + t
                    nc.tensor.matmul(
                        nrmPs[pj][rows, :],
                        lhsT=ones_strip[:, 31 - c : 63 - c],
                        rhs=pst[:, tl * W : (tl + 1) * W],
                        start=(pb % 2 == 0) and (t == 0),
                        stop=(pb % 2 == 1) and (t == NW - 1),
                        skip_group_check=True,
                        tile_position=(0, 32 * pj),
                    )
                if (pb % 2 == 1) and (ps == NST - 1):
                    nc.vector.tensor_copy(D_sb[rows, :], dotPs[pj][rows, :])
                    nc.vector.tensor_copy(NS_sb[rows, :], nrmPs[pj][rows, :])

            for b in range(BL):
                j = b // 2
                if b % 2 == 0:
                    dotPs[j] = psA.tile([128, W], F32, tag="dotP", name=f"dotP_{j}")
                    nrmPs[j] = psA.tile([128, W], F32, tag="nrmP", name=f"nrmP_{j}")
                for s in range(NST):
                    if (b, s) in stripes:
                        st = stripes[(b, s)]
                    else:
                        st = memp.tile([128, SW], BF16, tag="mst", name=f"mst_{b}_{s}")
                        nc.sync.dma_start(st[:], memT_ap[b][:, s * SW : (s + 1) * SW])
                    rows = slice(32 * j, 32 * j + 32)
                    for tl in range(SW // W):
                        t = s * (SW // W) + tl
                        c = NW * (b % 2) + t
                        nc.tensor.matmul(
                            dotPs[j][rows, :],
                            lhsT=strips[:, b, 31 - c : 63 - c],
                            rhs=st[:, tl * W : (tl + 1) * W],
                            start=(b % 2 == 0) and (t == 0),
                            stop=(b % 2 == 1) and (t == NW - 1),
                            skip_group_check=True,
                            tile_position=(0, 32 * j),
                        )
                    tch = cp.tile([128, 1], F32, name=f"tch_{b}_{s}")
                    if s % 2 == 0:
                        nc.scalar.copy(tch[:], st[:, 0:1])
                        nc.scalar.activation(st[:], st[:], AF.Square)
                    else:
                        nc.vector.tensor_copy(tch[:], st[:, 0:1])
                        nc.vector.tensor_mul(st[:], st[:], st[:])
                    if pending is not None:
                        emit_nrms(pending)
                    pending = (b, s, st)
            emit_nrms(pending)

            # ---- light phase ----
            # 1/(kn*mn) = exp(-0.5*ln(kn2*ns)); ln/exp LUTs are accurate
            # (~1e-5 rel) and stay in the natural_log_exp table set, so the
            # whole kernel uses ONE ACT table set (square/copy are in all).
            Lv = lp.tile([128, W], F32)
            nc.scalar.activation(Lv[:], NS_sb[:], AF.Ln, scale=F_kn2[:])
            y1 = lp.tile([128, W], F32)
            nc.scalar.activation(y1[:], Lv[:], AF.Exp, scale=-0.5)
            sim = lp.tile([128, W], F32)
            nc.vector.tensor_mul(sim[:], D_sb[:], y1[:])

            # per-batch scalars: beta, 1-gate, s0, s1, s2, gamma  (BL, 6)
            # projX = proj + 0*NS: artificial dep so the scheduler cannot
            # hoist these ACT transcendentals into the heavy-square stream
            # (that would thrash the ACT table set between squares)
            projX = lp.tile([BL, 6], F32)
            nc.vector.scalar_tensor_tensor(
                projX[:], NS_sb[0:BL, 0:6], 0.0, proj[:], op0=OP.mult, op1=OP.add
            )
            proj = projX
            scal = lp.tile([BL, 6], F32)
            # softplus(x) = ln(1 + exp(x)); beta = softplus + 1
            eb = lp.tile([BL, 1], F32)
            nc.scalar.activation(eb[:], proj[:, 0:1], AF.Exp)
            sp_b = lp.tile([BL, 1], F32)
            nc.scalar.activation(sp_b[:], eb[:], AF.Ln, bias=1.0)
            nc.vector.tensor_scalar_add(scal[:, 0:1], sp_b[:], 1.0)
            # gate = sigmoid(x) = 1 / (1 + exp(-x))
            eg = lp.tile([BL, 1], F32)
            nc.scalar.activation(eg[:], proj[:, 1:2], AF.Exp, scale=-1.0)
            dg = lp.tile([BL, 1], F32)
            nc.vector.tensor_scalar_add(dg[:], eg[:], 1.0)
            gate = lp.tile([BL, 1], F32)
            nc.vector.reciprocal(gate[:], dg[:])
            nc.vector.tensor_scalar(
                scal[:, 1:2], gate[:], -1.0, 1.0, op0=OP.mult, op1=OP.add
            )
            e3 = lp.tile([BL, 3], F32)
            nc.scalar.activation(e3[:], proj[:, 2:5], AF.Exp)
            ssum = lp.tile([BL, 1], F32)
            nc.vector.reduce_sum(ssum[:], e3[:], axis=mybir.AxisListType.X)
            rssum = lp.tile([BL, 1], F32)
            nc.vector.reciprocal(rssum[:], ssum[:])
            # on ACT: e3 is ACT-made, rssum DVE-made; a DVE tensor_scalar
            # would need two sync waits (TS struct supports one). Copy back
            # via DVE so scal stays single-producer for the FB matmul.
            sh3 = lp.tile([BL, 3], F32)
            nc.scalar.mul(sh3[:], e3[:], rssum[:])
            nc.vector.tensor_copy(scal[:, 2:5], sh3[:])
            # gamma = softplus(z) + 1 = ln(1 + exp(z)) + 1
            egm = lp.tile([BL, 1], F32)
            nc.scalar.activation(egm[:], proj[:, 5:6], AF.Exp)
            sp_g = lp.tile([BL, 1], F32)
            nc.scalar.activation(sp_g[:], egm[:], AF.Ln, bias=1.0)
            nc.vector.tensor_scalar_add(scal[:, 5:6], sp_g[:], 1.0)
            # broadcast to per-partition fields (128, 6)
            FB_ps = psB.tile([128, 6], F32, tag="tiny")
            nc.tensor.matmul(FB_ps[:], lhsT=indT_sb[:], rhs=scal[:], start=True, stop=True)
            FB = lp.tile([128, 6], F32)
            nc.vector.tensor_copy(FB[:], FB_ps[:])
            F_beta = FB[:, 0:1]
            F_g1 = FB[:, 1:2]
            F_s0 = FB[:, 2:3]
            F_s1 = FB[:, 3:4]
            F_s2 = FB[:, 4:5]
            F_gamma = FB[:, 5:6]

            # content weights: E = exp(beta * sim) (no max-sub: |beta*sim| small)
            E = lp.tile([128, W], F32)
            rs1 = lp.tile([128, 1], F32)
            nc.scalar.activation(E[:], sim[:], AF.Exp, scale=F_beta, accum_out=rs1[:])
            S_ps = psB.tile([BL, 1], F32, tag="tiny")
            nc.tensor.matmul(S_ps[:], lhsT=ind_sb[:], rhs=rs1[:], start=True, stop=True)
            Scol = lp.tile([BL, 1], F32)
            nc.vector.tensor_copy(Scol[:], S_ps[:])
            rS = lp.tile([BL, 1], F32)
            nc.vector.reciprocal(rS[:], Scol[:])
            gs = lp.tile([BL, 1], F32)
            nc.vector.tensor_mul(gs[:], gate[:], rS[:])
            F2_ps = psB.tile([128, 1], F32, tag="tiny")
            nc.tensor.matmul(F2_ps[:], lhsT=indT_sb[:], rhs=gs[:], start=True, stop=True)
            F_gs = lp.tile([128, 1], F32)
            nc.vector.tensor_copy(F_gs[:], F2_ps[:])

            # gated = gs*E + (1-gate)*pw   (gs = gate/softmax_sum)
            t4 = lp.tile([128, W], F32)
            nc.vector.tensor_scalar_mul(t4[:], pw_sb[:], F_g1)
            Esc = lp.tile([128, 1], F32)
            nc.vector.tensor_copy(Esc[:], E[:, 0:1])  # DVE observes ACT@E
            G = lp.tile([128, W], F32)
            nc.vector.scalar_tensor_tensor(
                G[:], E[:], F_gs[:], t4[:], op0=OP.mult, op1=OP.add
            )

            # circular conv: SH = s1*G + s0*roll(G,-1) + s2*roll(G,+1)
            SH = lp.tile([128, W], F32)
            nc.vector.tensor_scalar_mul(SH[:], G[:], F_s1)
            nc.vector.scalar_tensor_tensor(
                SH[:, 0 : W - 1], G[:, 1:W], F_s0, SH[:, 0 : W - 1],
                op0=OP.mult, op1=OP.add,
            )
            nc.vector.scalar_tensor_tensor(
                SH[:, 1:W], G[:, 0 : W - 1], F_s2, SH[:, 1:W],
                op0=OP.mult, op1=OP.add,
            )
            # boundary columns via PE transpose (partition shift is not a DVE op)
            # left-shift boundary: SH[p, W-1] += s0 * G[p+1 (wrap in batch), 0]
            rowL_ps = psB.tile([1, 128], F32, tag="tiny")
            nc.tensor.matmul(rowL_ps[:], lhsT=G[:, 0:1], rhs=ident[:], start=True, stop=True)
            rowL = lp.tile([1, 128], F32)
            nc.vector.tensor_copy(rowL[:], rowL_ps[:])
            rowLs = lp.tile([1, 128], F32)
            nc.vector.tensor_copy(rowLs[:, 0:127], rowL[:, 1:128])
            rL_v = rowL.rearrange("o (g s) -> o g s", s=16)
            rLs_v = rowLs.rearrange("o (g s) -> o g s", s=16)
            nc.vector.tensor_copy(rLs_v[:, :, 15:16], rL_v[:, :, 0:1])
            bl_ps = psB.tile([128, 1], F32, tag="tiny")
            nc.tensor.matmul(bl_ps[:], lhsT=rowLs[:], rhs=one1[:], start=True, stop=True)
            bl = lp.tile([128, 1], F32)
            nc.vector.tensor_copy(bl[:], bl_ps[:])
            nc.vector.scalar_tensor_tensor(
                SH[:, W - 1 : W], bl[:], F_s0, SH[:, W - 1 : W],
                op0=OP.mult, op1=OP.add,
            )
            # right-shift boundary: SH[p, 0] += s2 * G[p-1 (wrap in batch), W-1]
            rowR_ps = psB.tile([1, 128], F32, tag="tiny")
            nc.tensor.matmul(rowR_ps[:], lhsT=G[:, W - 1 : W], rhs=ident[:], start=True, stop=True)
            rowR = lp.tile([1, 128], F32)
            nc.vector.tensor_copy(rowR[:], rowR_ps[:])
            rowRs = lp.tile([1, 128], F32)
            nc.vector.tensor_copy(rowRs[:, 1:128], rowR[:, 0:127])
            rR_v = rowR.rearrange("o (g s) -> o g s", s=16)
            rRs_v = rowRs.rearrange("o (g s) -> o g s", s=16)
            nc.vector.tensor_copy(rRs_v[:, :, 0:1], rR_v[:, :, 15:16])
            br_ps = psB.tile([128, 1], F32, tag="tiny")
            nc.tensor.matmul(br_ps[:], lhsT=rowRs[:], rhs=one1[:], start=True, stop=True)
            br = lp.tile([128, 1], F32)
            nc.vector.tensor_copy(br[:], br_ps[:])
            nc.vector.scalar_tensor_tensor(
                SH[:, 0:1], br[:], F_s2, SH[:, 0:1], op0=OP.mult, op1=OP.add
            )

            # sharpening: P2 = (SH + 1e-8)^gamma = exp(gamma * ln(SH + 1e-8))
            Lg = lp.tile([128, W], F32)
            nc.scalar.activation(Lg[:], SH[:], AF.Ln, bias=eps_col[:])
            P2 = lp.tile([128, W], F32)
            rs2 = lp.tile([128, 1], F32)
            nc.scalar.activation(P2[:], Lg[:], AF.Exp, scale=F_gamma, accum_out=rs2[:])
            S2_ps = psB.tile([BL, 1], F32, tag="tiny")
            nc.tensor.matmul(S2_ps[:], lhsT=ind_sb[:], rhs=rs2[:], start=True, stop=True)
            S2 = lp.tile([BL, 1], F32)
            nc.vector.tensor_scalar_add(S2[:], S2_ps[:], EPS)
            r2 = lp.tile([BL, 1], F32)
            nc.vector.reciprocal(r2[:], S2[:])
            F3_ps = psB.tile([128, 1], F32, tag="tiny")
            nc.tensor.matmul(F3_ps[:], lhsT=indT_sb[:], rhs=r2[:], start=True, stop=True)
            F_r2 = lp.tile([128, 1], F32)
            nc.vector.tensor_copy(F_r2[:], F3_ps[:])

            P2sc = lp.tile([128, 1], F32)
            nc.vector.tensor_copy(P2sc[:], P2[:, 0:1])  # DVE observes ACT@P2
            outsb = lp.tile([128, W], F32)
            nc.vector.tensor_scalar_mul(outsb[:], P2[:], F_r2[:])
            nc.sync.dma_start(
                out_d[:].rearrange("b (q f) -> (b q) f", f=W), outsb[:]
            )
    nc.compile()
    return nc


def _get_nc():
    global _NC
    if _NC is None:
        _NC = build_nc()
    return _NC


def _enable_profiling():
    """Install the axon NTFF profile hook; the agent image lacks
    antenv.axon_hooks, so shim it and register the ctypes-based hook."""
    import types

    import concourse.bass_utils as bu

    bu.upload_artifacts = lambda tmpdir: tmpdir  # no artifact bucket here
    try:
        from antenv.axon_hooks import get_axon_ntff_profile_hook  # noqa: F401

        return
    except ImportError:
        pass
    import antenv

    mod = types.ModuleType("antenv.axon_hooks")
    _holder = {}
    mod.set_axon_ntff_profile_hook = lambda h: _holder.__setitem__("h", h)
    mod.get_axon_ntff_profile_hook = lambda: _holder.get("h")
    sys.modules["antenv.axon_hooks"] = mod
    antenv.axon_hooks = mod
    from trn_agent_boot.trn_boot import _ntff_profile_via_ctypes

    mod.set_axon_ntff_profile_hook(
        _ntff_profile_via_ctypes("/opt/axon/libaxon_pjrt.so")
    )


def kernel(**inputs):
    global LAST_RESULTS
    mem = np.ascontiguousarray(np.asarray(inputs["memory"], dtype=np.float32))
    cs = np.ascontiguousarray(np.asarray(inputs["controller_state"], dtype=np.float32))
    pw = np.ascontiguousarray(np.asarray(inputs["previous_weights"], dtype=np.float32))
    Wk = np.ascontiguousarray(np.asarray(inputs["Wk"], dtype=np.float32))
    Wcat = np.ascontiguousarray(
        np.concatenate(
            [
                np.asarray(inputs["Wb"], np.float32),
                np.asarray(inputs["Wg"], np.float32),
                np.asarray(inputs["Ws"], np.float32),
                np.asarray(inputs["Wgam"], np.float32),
            ],
            axis=1,
        )
    )
    brow = np.concatenate(
        [
            np.asarray(inputs["bb"], np.float32),
            np.asarray(inputs["bg"], np.float32),
            np.asarray(inputs["bs"], np.float32),
            np.asarray(inputs["bgam"], np.float32),
        ]
    )
    bias6 = np.ascontiguousarray(np.broadcast_to(brow[None, :], (BL, 6)).astype(np.float32))

    # shard: core c gets batches [c*BL, (c+1)*BL); memory pre-transposed to (BL, D, N)
    memT = np.ascontiguousarray(
        mem.reshape(NCORES, BL, N, D).transpose(0, 1, 3, 2)
    )
    import ml_dtypes
    memT = memT.astype(ml_dtypes.bfloat16)
    csT = np.ascontiguousarray(cs.reshape(NCORES, BL, C).transpose(0, 2, 1))
    pw_sh = pw.reshape(NCORES, BL, N)

    in_maps = [
        {
            "memT": memT[c],
            "csT": csT[c],
            "pw": np.ascontiguousarray(pw_sh[c]),
            "Wk": Wk,
            "Wcat": Wcat,
            "bias6": bias6,
        }
        for c in range(NCORES)
    ]
    nc = _get_nc()
    if PROFILE:
        _enable_profiling()
    res = run_bass_kernel_spmd(nc, in_maps, list(range(NCORES)), trace=PROFILE)
    LAST_RESULTS = res
    out = np.concatenate([r["out"] for r in res.results], axis=0)
    return out.astype(np.float32)


# revision 47
# speedup vs baseline: 1.1541x; 1.0810x over previous
"""Trainium2 Bass kernel for a differentiable addressing head (NTM-style).

Computes, for each batch b:
    key   = cs @ Wk;  beta = softplus(cs@Wb+bb)+1;  gate = sigmoid(cs@Wg+bg)
    shift = softmax(cs@Ws+bs);  gamma = softplus(cs@Wgam+bgam)+1
    sim   = (key . mem[n]) / (|key||mem[n]| + eps)
    cw    = softmax(beta * sim);  g = gate*cw + (1-gate)*pw
    sh    = circular_conv(g, shift);  w = (sh+1e-8)^gamma / (sum + eps)

Sharding: data-parallel over batch across 8 cores (8 batches/core).
Heavy phase is one pass over memory (32 MB/core) computing dot products and
row norms on the PE; everything else is O(B*N) light work.

Self-contained: hardcodes shapes B=64, N=8192, D=128, C=256.
"""

import os
import sys

import numpy as np

for _p in ("/opt/trn_rl_repo", "/opt/pypackages"):
    if _p not in sys.path and os.path.isdir(_p):
        sys.path.insert(0, _p)

import concourse.bacc as bacc
import concourse.bass as bass
import concourse.tile as tile
from concourse import mybir
from concourse.bass_utils import run_bass_kernel_spmd

F32 = mybir.dt.float32
F32R = mybir.dt.float32r  # single-pass truncated-precision PE mode (4x fp32)
BF16 = mybir.dt.bfloat16
AF = mybir.ActivationFunctionType
OP = mybir.AluOpType

B, N, D, C = 64, 8192, 128, 256
NCORES = 8
BL = B // NCORES          # batches per core = 8
NW = 16                   # 512-wide windows per batch row
W = N // NW               # 512, window width (= light-tile free dim)
NST = 2                   # DMA stripes per batch
SW = N // NST             # 2048, stripe width
EPS = 1e-8

# Module-level caches so repeat calls don't rebuild/recompile.
_NC = None
PROFILE = False
LAST_RESULTS = None       # BassKernelResults of the last run (for profiling)


def _consts():
    ind = np.zeros((128, BL), np.float32)        # ind[p, b] = 1 iff p//16 == b
    for p in range(128):
        ind[p, p // NW] = 1.0
    indT = np.ascontiguousarray(ind.T)           # (8, 128)
    ones_strip = np.zeros((128, 63), np.float32)  # ones column at col 31
    ones_strip[:, 31] = 1.0
    return ind, indT, ones_strip


def _patch_act_tables():
    """Prefer the combined natural_log_exp table set. The set chooser maps
    each activation to the FIRST set containing its func, so exp->set0 and
    ln->natural_log oscillate, costing a 1.3us table load per transition on
    the critical path. natural_log_exp_and_others covers every func this
    kernel uses (exp, ln, square, copy, identity) -> one load total."""
    import concourse.hw_specs as hw_specs

    if getattr(bacc, "_act_tables_patched", False):
        return
    orig = bacc.get_activation_tables

    def filtered(module_arch):
        t = orig(module_arch)
        pref = "natural_log_exp_and_others"
        if pref in t:
            mine = {
                AF.Exp, AF.Ln, AF.Square, AF.Copy, AF.Identity, AF.MemsetZero
            } & t[pref]
            # keep dict ORDER (ids are positional); only shrink other sets
            for k in t:
                if k != pref:
                    t[k] = t[k] - mine
        return t

    bacc.get_activation_tables = filtered
    bacc._act_tables_patched = True


def build_nc():
    # Bacc (not plain Bass): its compile() runs generate_event_semaphores,
    # which splits multi-wait sync conditions to satisfy the TRN2 limit of
    # one sync wait per instruction.
    _patch_act_tables()
    nc = bacc.Bacc()
    ind_np, indT_np, ones_strip_np = _consts()

    memT_d = nc.dram_tensor("memT", [BL, D, N], BF16, kind="ExternalInput")
    csT_d = nc.dram_tensor("csT", [C, BL], F32, kind="ExternalInput")
    pw_d = nc.dram_tensor("pw", [BL, N], F32, kind="ExternalInput")
    Wk_d = nc.dram_tensor("Wk", [C, D], F32, kind="ExternalInput")
    Wcat_d = nc.dram_tensor("Wcat", [C, 6], F32, kind="ExternalInput")
    bias6_d = nc.dram_tensor("bias6", [BL, 6], F32, kind="ExternalInput")
    out_d = nc.dram_tensor("out", [BL, N], F32, kind="ExternalOutput")

    ident_c = nc.inline_tensor(np.eye(128, dtype=np.float32), "ident_c")
    ind_c = nc.inline_tensor(ind_np, "ind_c")
    indT_c = nc.inline_tensor(indT_np, "indT_c")
    ones_col_c = nc.inline_tensor(np.ones((128, 1), np.float32), "ones_col_c")
    one1_c = nc.inline_tensor(np.ones((1, 1), np.float32), "one1_c")
    ones_strip_c = nc.inline_tensor(ones_strip_np, "ones_strip_c")
    eps_col_c = nc.inline_tensor(np.full((128, 1), EPS, np.float32), "eps_col_c")

    with tile.TileContext(nc) as tc:
        with (
            tc.tile_pool(name="const", bufs=1) as cp,
            tc.tile_pool(name="mem", bufs=6) as memp,
            tc.tile_pool(name="light", bufs=1) as lp,
            tc.tile_pool(name="psmm", bufs=2, space="PSUM") as psA,
            tc.tile_pool(name="pstiny", bufs=3, space="PSUM") as psB,
        ):
            # ---- constants / small inputs to SBUF ----
            # Self-loading fp32 matmuls only support ONE sync wait, and
            # Tile's vector clock is not transitive across procs. So every
            # tile a matmul reads is staged through GPSIMD (one producer
            # proc); once PE has waited on the newest GPSIMD tick, all
            # const reads are free.
            def dma_then_g(name, shape, src_ap, dtype=F32):
                raw = cp.tile(shape, F32, name=f"{name}_raw")
                nc.sync.dma_start(raw[:], src_ap)
                g = cp.tile(shape, dtype, name=name)
                nc.gpsimd.tensor_copy(g[:], raw[:])
                return g

            csT0 = dma_then_g("csT0", [128, BL], csT_d[:][0:128, :])
            csT1 = dma_then_g("csT1", [128, BL], csT_d[:][128:256, :])
            Wk0 = dma_then_g("Wk0", [128, D], Wk_d[:][0:128, :])
            Wk1 = dma_then_g("Wk1", [128, D], Wk_d[:][128:256, :])
            memT_ap = memT_d[:]
            stripes = {}
            for s in range(NST):
                st = memp.tile([128, SW], BF16, tag="mst", name=f"mst_0_{s}")
                nc.sync.dma_start(st[:], memT_ap[0][:, s * SW : (s + 1) * SW])
                stripes[(0, s)] = st
            Wc0 = dma_then_g("Wc0", [128, 6], Wcat_d[:][0:128, :])
            Wc1 = dma_then_g("Wc1", [128, 6], Wcat_d[:][128:256, :])
            ident = dma_then_g("ident", [128, 128], ident_c[:])
            ind_sb = dma_then_g("ind_sb", [128, BL], ind_c[:])
            indT_sb = dma_then_g("indT_sb", [BL, 128], indT_c[:])
            ones_col = dma_then_g("ones_col", [128, 1], ones_col_c[:])
            one1 = dma_then_g("one1", [1, 1], one1_c[:])
            ones_strip = dma_then_g("ones_strip", [128, 63], ones_strip_c[:], dtype=BF16)
            bias6 = cp.tile([BL, 6], F32)
            nc.sync.dma_start(bias6[:], bias6_d[:])
            eps_col = cp.tile([128, 1], F32)
            nc.sync.dma_start(eps_col[:], eps_col_c[:])
            pw_raw = cp.tile([128, W], F32)
            nc.sync.dma_start(pw_raw[:], pw_d[:].rearrange("b (q f) -> (b q) f", f=W))
            pw_sb = cp.tile([128, W], F32)
            nc.vector.tensor_copy(pw_sb[:], pw_raw[:])

            # ---- projections: key_T (D, BL) and proj (BL, 6) ----
            key_ps = psB.tile([128, BL], F32, tag="tiny")
            nc.tensor.matmul(key_ps[:], lhsT=Wk0[:], rhs=csT0[:], start=True, stop=False)
            nc.tensor.matmul(key_ps[:], lhsT=Wk1[:], rhs=csT1[:], start=False, stop=True)
            keyT = cp.tile([128, BL], F32)
            nc.vector.tensor_copy(keyT[:], key_ps[:])

            proj_ps = psB.tile([BL, 6], F32, tag="tiny")
            nc.tensor.matmul(proj_ps[:], lhsT=csT0[:], rhs=Wc0[:], start=True, stop=False)
            nc.tensor.matmul(proj_ps[:], lhsT=csT1[:], rhs=Wc1[:], start=False, stop=True)
            proj = lp.tile([BL, 6], F32)
            nc.vector.tensor_add(proj[:], proj_ps[:], bias6[:])

            # zero-padded key strips: strips[:, b, 31] = key_T[:, b]
            # (all on DVE: gpsimd runs on parallel Q7 lanes, so mixed-engine
            # writers would force multi-wait copies, which walrus rejects)
            strips = cp.tile([128, BL, 63], BF16)
            nc.vector.memset(strips[:], 0.0)
            for b in range(BL):
                nc.vector.tensor_copy(strips[:, b, 31:32], keyT[:, b : b + 1])

            # Absorber matmuls: advance PE's observed GPSIMD tick past every
            # staged constant so later matmuls carry at most one new wait
            # (self-loading fp32 matmuls support only a single sync wait).
            # The output tile is write-only and never rotated, so its slot
            # release never puts a wait on another instruction.
            absorb = psB.tile([128, 8], F32, tag="absorb", bufs=1, name="absorb")
            nc.tensor.matmul(absorb[:, 0:1], lhsT=ident[:], rhs=ones_col[:],
                             start=True, stop=True, skip_group_check=True)
            nc.tensor.matmul(absorb[0:8, 1:2], lhsT=ind_sb[:], rhs=ones_col[:],
                             start=True, stop=True, skip_group_check=True)
            nc.tensor.matmul(absorb[0:128, 2:3], lhsT=indT_sb[:], rhs=indT_sb[:, 0:1],
                             start=True, stop=True, skip_group_check=True)
            nc.tensor.matmul(absorb[0:1, 3:4], lhsT=one1[:], rhs=one1[:],
                             start=True, stop=True, skip_group_check=True)
            nc.tensor.matmul(absorb[0:63, 4:5], lhsT=ones_strip[:], rhs=ones_strip[:, 31:32],
                             start=True, stop=True, skip_group_check=True)
            nc.tensor.matmul(absorb[0:8, 5:6], lhsT=strips[:, :, 31], rhs=strips[:, 0, 31:32],
                             start=True, stop=True, skip_group_check=True)

            # |key|^2 per batch -> (BL, 1)
            kq = lp.tile([128, BL], F32)
            nc.scalar.activation(kq[:], keyT[:], AF.Square)
            kn2_ps = psB.tile([BL, 1], F32, tag="tiny")
            nc.tensor.matmul(kn2_ps[:], lhsT=kq[:], rhs=ones_col[:], start=True, stop=True)
            kn2 = lp.tile([BL, 1], F32)
            nc.vector.tensor_copy(kn2[:], kn2_ps[:])
            kn2F_ps = psB.tile([128, 1], F32, tag="tiny")
            nc.tensor.matmul(kn2F_ps[:], lhsT=indT_sb[:], rhs=kn2[:], start=True, stop=True)
            F_kn2 = lp.tile([128, 1], F32)
            nc.vector.tensor_copy(F_kn2[:], kn2F_ps[:])

            # ---- heavy phase: dot[b, n] and normsq[b, n] ----
            D_sb = lp.tile([128, W], F32)    # dot, light layout (p = b*16+t, f)
            NS_sb = lp.tile([128, W], F32)   # |mem|^2, light layout
            # Per stripe: 4 dot matmuls read the raw stripe, then the stripe
            # is squared IN PLACE (slot-reuse waits land on the exempt DMA
            # instruction), then the PREVIOUS stripe's 4 norm matmuls run —
            # one-stripe software pipeline so the PE never waits on a square.
            # A tiny fresh-tile "toucher" copy absorbs the stripe's DMA tick
            # into the squaring engine's clock first, keeping every compute
            # instruction at <=1 sync wait (walrus codegen limit).
            pending = []
            dotPs = {}
            nrmPs = {}

            def emit_nrms(p):
                pb, ps, pst = p
                pj = pb // 2
                rows = slice(32 * pj, 32 * pj + 32)
                for tl in range(SW // W):
                    t = ps * (SW // W) + tl
                    c = NW * (pb % 2) + t
                    nc.tensor.matmul(
                        nrmPs[pj][rows, :],
                        lhsT=ones_strip[:, 31 - c : 63 - c],
                        rhs=pst[:, tl * W : (tl + 1) * W],
                        start=(pb % 2 == 0) and (t == 0),
                        stop=(pb % 2 == 1) and (t == NW - 1),
                        skip_group_check=True,
                        tile_position=(0, 32 * pj),
                    )
                if (pb % 2 == 1) and (ps == NST - 1):
                    nc.vector.tensor_copy(D_sb[rows, :], dotPs[pj][rows, :])
                    nc.vector.tensor_copy(NS_sb[rows, :], nrmPs[pj][rows, :])

            for b in range(BL):
                j = b // 2
                if b % 2 == 0:
                    dotPs[j] = psA.tile([128, W], F32, tag="dotP", name=f"dotP_{j}")
                    nrmPs[j] = psA.tile([128, W], F32, tag="nrmP", name=f"nrmP_{j}")
                for s in range(NST):
                    if (b, s) in stripes:
                        st = stripes[(b, s)]
                    else:
                        st = memp.tile([128, SW], BF16, tag="mst", name=f"mst_{b}_{s}")
                        nc.sync.dma_start(st[:], memT_ap[b][:, s * SW : (s + 1) * SW])
                    rows = slice(32 * j, 32 * j + 32)
                    for tl in range(SW // W):
                        t = s * (SW // W) + tl
                        c = NW * (b % 2) + t
                        nc.tensor.matmul(
                            dotPs[j][rows, :],
                            lhsT=strips[:, b, 31 - c : 63 - c],
                            rhs=st[:, tl * W : (tl + 1) * W],
                            start=(b % 2 == 0) and (t == 0),
                            stop=(b % 2 == 1) and (t == NW - 1),
                            skip_group_check=True,
                            tile_position=(0, 32 * j),
                        )
                    tch = cp.tile([128, 1], F32, name=f"tch_{b}_{s}")
                    if s % 2 == 0:
                        nc.scalar.copy(tch[:], st[:, 0:1])
                        nc.scalar.activation(st[:], st[:], AF.Square)
                    else:
                        nc.vector.tensor_copy(tch[:], st[:, 0:1])
                        nc.vector.tensor_mul(st[:], st[:], st[:])
                    pending.append((b, s, st))
                    if len(pending) > 2:
                        emit_nrms(pending.pop(0))
            for p in pending:
                emit_nrms(p)

            # ---- light phase ----
            # 1/(kn*mn) = exp(-0.5*ln(kn2*ns)); ln/exp LUTs are accurate
            # (~1e-5 rel) and stay in the natural_log_exp table set, so the
            # whole kernel uses ONE ACT table set (square/copy are in all).
            Lv = lp.tile([128, W], F32)
            nc.scalar.activation(Lv[:], NS_sb[:], AF.Ln, scale=F_kn2[:])
            y1 = lp.tile([128, W], F32)
            nc.scalar.activation(y1[:], Lv[:], AF.Exp, scale=-0.5)
            sim = lp.tile([128, W], F32)
            nc.vector.tensor_mul(sim[:], D_sb[:], y1[:])

            # per-batch scalars: beta, 1-gate, s0, s1, s2, gamma  (BL, 6)
            # projX = proj + 0*NS: artificial dep so the scheduler cannot
            # hoist these ACT transcendentals into the heavy-square stream
            # (that would thrash the ACT table set between squares)
            projX = lp.tile([BL, 6], F32)
            nc.vector.scalar_tensor_tensor(
                projX[:], NS_sb[0:BL, 0:6], 0.0, proj[:], op0=OP.mult, op1=OP.add
            )
            proj = projX
            scal = lp.tile([BL, 6], F32)
            # softplus(x) = ln(1 + exp(x)); beta = softplus + 1
            eb = lp.tile([BL, 1], F32)
            nc.scalar.activation(eb[:], proj[:, 0:1], AF.Exp)
            sp_b = lp.tile([BL, 1], F32)
            nc.scalar.activation(sp_b[:], eb[:], AF.Ln, bias=1.0)
            nc.vector.tensor_scalar_add(scal[:, 0:1], sp_b[:], 1.0)
            # gate = sigmoid(x) = 1 / (1 + exp(-x))
            eg = lp.tile([BL, 1], F32)
            nc.scalar.activation(eg[:], proj[:, 1:2], AF.Exp, scale=-1.0)
            dg = lp.tile([BL, 1], F32)
            nc.vector.tensor_scalar_add(dg[:], eg[:], 1.0)
            gate = lp.tile([BL, 1], F32)
            nc.vector.reciprocal(gate[:], dg[:])
            nc.vector.tensor_scalar(
                scal[:, 1:2], gate[:], -1.0, 1.0, op0=OP.mult, op1=OP.add
            )
            e3 = lp.tile([BL, 3], F32)
            nc.scalar.activation(e3[:], proj[:, 2:5], AF.Exp)
            ssum = lp.tile([BL, 1], F32)
            nc.vector.reduce_sum(ssum[:], e3[:], axis=mybir.AxisListType.X)
            rssum = lp.tile([BL, 1], F32)
            nc.vector.reciprocal(rssum[:], ssum[:])
            # on ACT: e3 is ACT-made, rssum DVE-made; a DVE tensor_scalar
            # would need two sync waits (TS struct supports one). Copy back
            # via DVE so scal stays single-producer for the FB matmul.
            sh3 = lp.tile([BL, 3], F32)
            nc.scalar.mul(sh3[:], e3[:], rssum[:])
            nc.vector.tensor_copy(scal[:, 2:5], sh3[:])
            # gamma = softplus(z) + 1 = ln(1 + exp(z)) + 1
            egm = lp.tile([BL, 1], F32)
            nc.scalar.activation(egm[:], proj[:, 5:6], AF.Exp)
            sp_g = lp.tile([BL, 1], F32)
            nc.scalar.activation(sp_g[:], egm[:], AF.Ln, bias=1.0)
            nc.vector.tensor_scalar_add(scal[:, 5:6], sp_g[:], 1.0)
            # broadcast to per-partition fields (128, 6)
            FB_ps = psB.tile([128, 6], F32, tag="tiny")
            nc.tensor.matmul(FB_ps[:], lhsT=indT_sb[:], rhs=scal[:], start=True, stop=True)
            FB = lp.tile([128, 6], F32)
            nc.vector.tensor_copy(FB[:], FB_ps[:])
            F_beta = FB[:, 0:1]
            F_g1 = FB[:, 1:2]
            F_s0 = FB[:, 2:3]
            F_s1 = FB[:, 3:4]
            F_s2 = FB[:, 4:5]
            F_gamma = FB[:, 5:6]

            # content weights: E = exp(beta * sim) (no max-sub: |beta*sim| small)
            E = lp.tile([128, W], F32)
            rs1 = lp.tile([128, 1], F32)
            nc.scalar.activation(E[:], sim[:], AF.Exp, scale=F_beta, accum_out=rs1[:])
            S_ps = psB.tile([BL, 1], F32, tag="tiny")
            nc.tensor.matmul(S_ps[:], lhsT=ind_sb[:], rhs=rs1[:], start=True, stop=True)
            Scol = lp.tile([BL, 1], F32)
            nc.vector.tensor_copy(Scol[:], S_ps[:])
            rS = lp.tile([BL, 1], F32)
            nc.vector.reciprocal(rS[:], Scol[:])
            gs = lp.tile([BL, 1], F32)
            nc.vector.tensor_mul(gs[:], gate[:], rS[:])
            F2_ps = psB.tile([128, 1], F32, tag="tiny")
            nc.tensor.matmul(F2_ps[:], lhsT=indT_sb[:], rhs=gs[:], start=True, stop=True)
            F_gs = lp.tile([128, 1], F32)
            nc.vector.tensor_copy(F_gs[:], F2_ps[:])

            # gated = gs*E + (1-gate)*pw   (gs = gate/softmax_sum)
            t4 = lp.tile([128, W], F32)
            nc.vector.tensor_scalar_mul(t4[:], pw_sb[:], F_g1)
            Esc = lp.tile([128, 1], F32)
            nc.vector.tensor_copy(Esc[:], E[:, 0:1])  # DVE observes ACT@E
            G = lp.tile([128, W], F32)
            nc.vector.scalar_tensor_tensor(
                G[:], E[:], F_gs[:], t4[:], op0=OP.mult, op1=OP.add
            )

            # circular conv: SH = s1*G + s0*roll(G,-1) + s2*roll(G,+1)
            SH = lp.tile([128, W], F32)
            nc.vector.tensor_scalar_mul(SH[:], G[:], F_s1)
            nc.vector.scalar_tensor_tensor(
                SH[:, 0 : W - 1], G[:, 1:W], F_s0, SH[:, 0 : W - 1],
                op0=OP.mult, op1=OP.add,
            )
            nc.vector.scalar_tensor_tensor(
                SH[:, 1:W], G[:, 0 : W - 1], F_s2, SH[:, 1:W],
                op0=OP.mult, op1=OP.add,
            )
            # boundary columns via PE transpose (partition shift is not a DVE op)
            # left-shift boundary: SH[p, W-1] += s0 * G[p+1 (wrap in batch), 0]
            rowL_ps = psB.tile([1, 128], F32, tag="tiny")
            nc.tensor.matmul(rowL_ps[:], lhsT=G[:, 0:1], rhs=ident[:], start=True, stop=True)
            rowL = lp.tile([1, 128], F32)
            nc.vector.tensor_copy(rowL[:], rowL_ps[:])
            rowLs = lp.tile([1, 128], F32)
            nc.vector.tensor_copy(rowLs[:, 0:127], rowL[:, 1:128])
            rL_v = rowL.rearrange("o (g s) -> o g s", s=16)
            rLs_v = rowLs.rearrange("o (g s) -> o g s", s=16)
            nc.vector.tensor_copy(rLs_v[:, :, 15:16], rL_v[:, :, 0:1])
            bl_ps = psB.tile([128, 1], F32, tag="tiny")
            nc.tensor.matmul(bl_ps[:], lhsT=rowLs[:], rhs=one1[:], start=True, stop=True)
            bl = lp.tile([128, 1], F32)
            nc.vector.tensor_copy(bl[:], bl_ps[:])
            nc.vector.scalar_tensor_tensor(
                SH[:, W - 1 : W], bl[:], F_s0, SH[:, W - 1 : W],
                op0=OP.mult, op1=OP.add,
            )
            # right-shift boundary: SH[p, 0] += s2 * G[p-1 (wrap in batch), W-1]
            rowR_ps = psB.tile([1, 128], F32, tag="tiny")
            nc.tensor.matmul(rowR_ps[:], lhsT=G[:, W - 1 : W], rhs=ident[:], start=True, stop=True)
            rowR = lp.tile([1, 128], F32)
            nc.vector.tensor_copy(rowR[:], rowR_ps[:])
            rowRs = lp.tile([1, 128], F32)
            nc.vector.tensor_copy(rowRs[:, 1:128], rowR[:, 0:127])
            rR_v = rowR.rearrange("o (g s) -> o g s", s=16)
            rRs_v = rowRs.rearrange("o (g s) -> o g s", s=16)
            nc.vector.tensor_copy(rRs_v[:, :, 0:1], rR_v[:, :, 15:16])
            br_ps = psB.tile([128, 1], F32, tag="tiny")
            nc.tensor.matmul(br_ps[:], lhsT=rowRs[:], rhs=one1[:], start=True, stop=True)
            br = lp.tile([128, 1], F32)
            nc.vector.tensor_copy(br[:], br_ps[:])
            nc.vector.scalar_tensor_tensor(
                SH[:, 0:1], br[:], F_s2, SH[:, 0:1], op0=OP.mult, op1=OP.add
            )

            # sharpening: P2 = (SH + 1e-8)^gamma = exp(gamma * ln(SH + 1e-8))
            Lg = lp.tile([128, W], F32)
            nc.scalar.activation(Lg[:], SH[:], AF.Ln, bias=eps_col[:])
            P2 = lp.tile([128, W], F32)
            rs2 = lp.tile([128, 1], F32)
            nc.scalar.activation(P2[:], Lg[:], AF.Exp, scale=F_gamma, accum_out=rs2[:])
            S2_ps = psB.tile([BL, 1], F32, tag="tiny")
            nc.tensor.matmul(S2_ps[:], lhsT=ind_sb[:], rhs=rs2[:], start=True, stop=True)
            S2 = lp.tile([BL, 1], F32)
            nc.vector.tensor_scalar_add(S2[:], S2_ps[:], EPS)
            r2 = lp.tile([BL, 1], F32)
            nc.vector.reciprocal(r2[:], S2[:])
            F3_ps = psB.tile([128, 1], F32, tag="tiny")
            nc.tensor.matmul(F3_ps[:], lhsT=indT_sb[:], rhs=r2[:], start=True, stop=True)
            F_r2 = lp.tile([128, 1], F32)
            nc.vector.tensor_copy(F_r2[:], F3_ps[:])

            P2sc = lp.tile([128, 1], F32)
            nc.vector.tensor_copy(P2sc[:], P2[:, 0:1])  # DVE observes ACT@P2
            outsb = lp.tile([128, W], F32)
            nc.vector.tensor_scalar_mul(outsb[:], P2[:], F_r2[:])
            nc.sync.dma_start(
                out_d[:].rearrange("b (q f) -> (b q) f", f=W), outsb[:]
            )
    nc.compile()
    return nc


def _get_nc():
    global _NC
    if _NC is None:
        _NC = build_nc()
    return _NC


def _enable_profiling():
    """Install the axon NTFF profile hook; the agent image lacks
    antenv.axon_hooks, so shim it and register the ctypes-based hook."""
    import types

    import concourse.bass_utils as bu

    bu.upload_artifacts = lambda tmpdir: tmpdir  # no artifact bucket here
    try:
        from antenv.axon_hooks import get_axon_ntff_profile_hook  # noqa: F401

        return
    except ImportError:
        pass
    import antenv

    mod = types.ModuleType("antenv.axon_hooks")
    _holder = {}
    mod.set_axon_ntff_profile_hook = lambda h: _holder.__setitem__("h", h)
    mod.get_axon_ntff_profile_hook = lambda: _holder.get("h")
    sys.modules["antenv.axon_hooks"] = mod
    antenv.axon_hooks = mod
    from trn_agent_boot.trn_boot import _ntff_profile_via_ctypes

    mod.set_axon_ntff_profile_hook(
        _ntff_profile_via_ctypes("/opt/axon/libaxon_pjrt.so")
    )


def kernel(**inputs):
    global LAST_RESULTS
    mem = np.ascontiguousarray(np.asarray(inputs["memory"], dtype=np.float32))
    cs = np.ascontiguousarray(np.asarray(inputs["controller_state"], dtype=np.float32))
    pw = np.ascontiguousarray(np.asarray(inputs["previous_weights"], dtype=np.float32))
    Wk = np.ascontiguousarray(np.asarray(inputs["Wk"], dtype=np.float32))
    Wcat = np.ascontiguousarray(
        np.concatenate(
            [
                np.asarray(inputs["Wb"], np.float32),
                np.asarray(inputs["Wg"], np.float32),
                np.asarray(inputs["Ws"], np.float32),
                np.asarray(inputs["Wgam"], np.float32),
            ],
            axis=1,
        )
    )
    brow = np.concatenate(
        [
            np.asarray(inputs["bb"], np.float32),
            np.asarray(inputs["bg"], np.float32),
            np.asarray(inputs["bs"], np.float32),
            np.asarray(inputs["bgam"], np.float32),
        ]
    )
    bias6 = np.ascontiguousarray(np.broadcast_to(brow[None, :], (BL, 6)).astype(np.float32))

    # shard: core c gets batches [c*BL, (c+1)*BL); memory pre-transposed to (BL, D, N)
    memT = np.ascontiguousarray(
        mem.reshape(NCORES, BL, N, D).transpose(0, 1, 3, 2)
    )
    import ml_dtypes
    memT = memT.astype(ml_dtypes.bfloat16)
    csT = np.ascontiguousarray(cs.reshape(NCORES, BL, C).transpose(0, 2, 1))
    pw_sh = pw.reshape(NCORES, BL, N)

    in_maps = [
        {
            "memT": memT[c],
            "csT": csT[c],
            "pw": np.ascontiguousarray(pw_sh[c]),
            "Wk": Wk,
            "Wcat": Wcat,
            "bias6": bias6,
        }
        for c in range(NCORES)
    ]
    nc = _get_nc()
    if PROFILE:
        _enable_profiling()
    res = run_bass_kernel_spmd(nc, in_maps, list(range(NCORES)), trace=PROFILE)
    LAST_RESULTS = res
    out = np.concatenate([r["out"] for r in res.results], axis=0)
    return out.astype(np.float32)
